# revision 10
# baseline (speedup 1.0000x reference)
"""Trainium2 Bass kernel for nn_Classifier0 (quadrant-sum classifier).

reference:
    agg[n, q]  = quadrant sums of x[n] (512x512, quadrants of 256x256)
    w          = g * v[..., 0] / ||v||            [4, 4]
    y          = agg[:, :, None] * w + b_fgl      [N, 4, 4]
    out        = y.reshape(N, 16) @ W_fc.T + b_fc [N, 10]

Algebraic refactor (exact in real arithmetic):
    out[n, c] = sum_q agg[n, q] * A[q, c] + cc[c]
      A[q, c] = sum_j w[q, j] * W_fc[c, 4q + j]         (4 x 10, host, fp64)
      cc[c]   = b_fgl.ravel() @ W_fc[c] + b_fc[c]       (10, host, fp64)

Device work (data-parallel, 32 samples per core):
  - bulk: 7 chunks of C=4 samples, one contiguous 4 MB DMA each into a
    [128, 8192] tile (partition p holds 16 consecutive image rows of
    sample p // 32; p % 32 < 16 is the image's top half).
  - DVE tensor_reduce sums the left 256 columns of each row, ACT
    (in-place activation Copy with accum_out) sums the right 256
    -> bufL/bufR [128, 7].
  - the bulk quadrant contraction + fc (3 PSUM-accumulated matmuls with
    zero-masked weights; psum [7, 40] row-major equals y[0:28] with
    n = 4k + j) is placed BEFORE the tail loop so it fires as soon as
    chunk 6 is reduced (~80% through the stream) and its y write is
    hidden under the x stream.  y writes go out on the scalar (ACT)
    HWDGE ring so they never block tail x DMAs in the SP ring's FIFO.
  - tail: samples 28..31 as 4 single-sample 1 MB chunks keep the
    critical tail short (last reduce covers 1 MB, not 4 MB).

Per-core stream is SBUF-fabric bound: 16 SDMA engines x ~26.5 GB/s
= ~424 GB/s -> 33.6 MB in ~79 us, plus a ~8.6 us fixed NEFF preamble
(engine init, instruction loads, first-descriptor/HBM latency) and a
postamble of per-event EVENT_SEMAPHORE resets whose length scales with
the instruction/event count -- which is why chunk count is kept low.
"""

import numpy as np

N, S = 256, 512
H = S // 2
NCORES = 8
SPC = N // NCORES  # samples per core (32)
NCLS = 10

C = 4  # samples per DMA chunk (bulk)
NCH = 7  # bulk chunks per core (samples 0..27)
NT = 4  # single-sample tail chunks (samples 28..31)
PPS = 128 // C  # partitions per sample in a bulk chunk (32)
RPP = S // PPS  # image rows per partition (16)
FREE = S * RPP  # floats per partition per bulk chunk (8192)
FREE1 = S * 4  # floats per partition per C=1 chunk (2048)

_PROGRAM_CACHE = {}


def _build_program():
    from contextlib import ExitStack

    import concourse.bacc as bacc
    import concourse.mybir as mybir
    import concourse.tile as tile

    nc = bacc.Bacc("TRN2", target_bir_lowering=False, debug=False)
    dt = mybir.dt.float32
    dth = mybir.dt.float16

    x_t = nc.dram_tensor("x", [NCH, 128, FREE], dt, kind="ExternalInput")
    x1_t = nc.dram_tensor("x1", [NT, 128, FREE1], dt, kind="ExternalInput")
    # all folded params packed into one tensor: cols 0:40 walm, 40:80 warm,
    # 80:90 walm1, 90:100 warm1; row 0 cols 100:140 ccbt, 140:150 ccbt1
    cst_t = nc.dram_tensor("cst", [128, 150], dt, kind="ExternalInput")
    y_t = nc.dram_tensor("y", [SPC, NCLS], dt, kind="ExternalOutput")

    with tile.TileContext(nc) as tc, ExitStack() as ctx:
        xpool = ctx.enter_context(tc.tile_pool(name="xp", bufs=4))
        tpool = ctx.enter_context(tc.tile_pool(name="tp", bufs=4))
        cpool = ctx.enter_context(tc.tile_pool(name="cp", bufs=1))
        ppool = ctx.enter_context(tc.tile_pool(name="pp", bufs=1, space="PSUM"))

        x_ap = x_t.ap()
        x1_ap = x1_t.ap()
        # first 28 y rows viewed as [7 chunks, 40]
        y2 = y_t.ap()[0 : C * NCH, :].rearrange("(k j) c -> k (j c)", j=C)

        bufL = cpool.tile([128, NCH], dt)
        bufR = cpool.tile([128, NCH], dt)
        bufL1 = cpool.tile([128, NT], dt)
        bufR1 = cpool.tile([128, NT], dt)
        # zero-padded lhsT buffers for the final 0.25 MB slice of sample 31:
        # only col NT-1 is written; cols 0..NT-2 stay zero so the psum
        # accumulation leaves rows 0..NT-2 untouched
        bufL2 = cpool.tile([128, NT], dt)
        bufR2 = cpool.tile([128, NT], dt)
        nc.vector.memset(bufL2[:], 0.0)
        nc.vector.memset(bufR2[:], 0.0)
        # one constant load on the scalar engine's HWDGE ring: the SP ring
        # starts streaming x immediately and GpSimd stays fully idle
        cst = cpool.tile([128, 150], dt)
        nc.scalar.dma_start(cst[:], cst_t.ap())
        walm, warm = cst[:, 0 : C * NCLS], cst[:, C * NCLS : 2 * C * NCLS]
        walm1, warm1 = cst[:, 80:90], cst[:, 90:100]
        ccbt, ccbt1 = cst[0:1, 100 : 100 + C * NCLS], cst[0:1, 140:150]
        ones1 = cpool.tile([1, NCH], dt)
        nc.vector.memset(ones1[:], 1.0)

        for k in range(NCH):
            xt = xpool.tile([128, FREE], dt)
            nc.sync.dma_start(xt[:], x_ap[k])
            xv = xt[:].rearrange("p (r c) -> p r c", c=S)
            nc.vector.tensor_reduce(
                bufL[:, k : k + 1],
                xv[:, :, 0:H],
                axis=mybir.AxisListType.XY,
                op=mybir.AluOpType.add,
            )
            nc.scalar.activation(
                xv[:, :, H:S],
                xv[:, :, H:S],
                mybir.ActivationFunctionType.Copy,
                accum_out=bufR[:, k : k + 1],
            )

        # bulk chunks: ready ~80% through the stream -- matmul + copy +
        # y write all hidden under the tail of the x stream
        psumA = ppool.tile([NCH, C * NCLS], dt)
        nc.tensor.matmul(psumA[:], lhsT=bufL[:], rhs=walm, start=True, stop=False)
        nc.tensor.matmul(psumA[:], lhsT=bufR[:], rhs=warm, start=False, stop=False)
        nc.tensor.matmul(psumA[:], lhsT=ones1[:], rhs=ccbt, start=False, stop=True)
        outA = cpool.tile([NCH, C * NCLS], dt)
        nc.vector.tensor_copy(outA[:], psumA[:])
        nc.scalar.dma_start(y2[:], outA[:])

        # single-sample tail chunks: half-size reduces on the critical tail
        for k in range(NT - 1):
            xt1 = tpool.tile([128, FREE1], dt)
            nc.sync.dma_start(xt1[:], x1_ap[k])
            xv1 = xt1[:].rearrange("p (r c) -> p r c", c=S)
            nc.vector.tensor_reduce(
                bufL1[:, k : k + 1],
                xv1[:, :, 0:H],
                axis=mybir.AxisListType.XY,
                op=mybir.AluOpType.add,
            )
            nc.scalar.activation(
                xv1[:, :, H:S],
                xv1[:, :, H:S],
                mybir.ActivationFunctionType.Copy,
                accum_out=bufR1[:, k : k + 1],
            )

        # last sample split 0.75 MB + 0.25 MB: the final reduce covers only
        # 512 floats/partition, shortening the critical tail.  Each partition
        # keeps its original 4-row range (rows 4p..4p+3), so top/bottom masks
        # are unchanged; the last row (slice b) lands in bufL2/bufR2 col NT-1.
        kl = NT - 1
        xt1 = tpool.tile([128, FREE1], dt)
        nc.sync.dma_start(xt1[:, 0 : 3 * S], x1_ap[kl][:, 0 : 3 * S])
        xva = xt1[:, 0 : 3 * S].rearrange("p (r c) -> p r c", c=S)
        nc.vector.tensor_reduce(
            bufL1[:, kl : kl + 1],
            xva[:, :, 0:H],
            axis=mybir.AxisListType.XY,
            op=mybir.AluOpType.add,
        )
        nc.scalar.activation(
            xva[:, :, H:S],
            xva[:, :, H:S],
            mybir.ActivationFunctionType.Copy,
            accum_out=bufR1[:, kl : kl + 1],
        )

        # tail samples 28..31: psum accumulation; everything not depending on
        # the last 0.25 MB slice is issued first
        ones2 = ones1[:, 0:NT]
        psumB = ppool.tile([NT, NCLS], dt)
        nc.tensor.matmul(psumB[:], lhsT=bufL1[:], rhs=walm1, start=True, stop=False)
        nc.tensor.matmul(psumB[:], lhsT=bufR1[:], rhs=warm1, start=False, stop=False)

        # both halves of the final 0.25 MB slice go to DVE (0.4 us each):
        # ACT still owes 31a's ACTIVATE+READ at this point, so routing 31b
        # through ACT would serialize behind it
        nc.sync.dma_start(xt1[:, 3 * S : FREE1], x1_ap[kl][:, 3 * S : FREE1])
        xvb = xt1[:, 3 * S : FREE1].rearrange("p (r c) -> p r c", c=S)
        nc.vector.tensor_reduce(
            bufL2[:, kl : kl + 1],
            xvb[:, :, 0:H],
            axis=mybir.AxisListType.XY,
            op=mybir.AluOpType.add,
        )
        nc.vector.tensor_reduce(
            bufR2[:, kl : kl + 1],
            xvb[:, :, H:S],
            axis=mybir.AxisListType.XY,
            op=mybir.AluOpType.add,
        )
        nc.tensor.matmul(psumB[:], lhsT=bufL2[:], rhs=walm1, start=False, stop=False)
        nc.tensor.matmul(psumB[:], lhsT=bufR2[:], rhs=warm1, start=False, stop=False)
        nc.tensor.matmul(psumB[:], lhsT=ones2, rhs=ccbt1, start=False, stop=True)
        outB = cpool.tile([NT, NCLS], dt)
        nc.vector.tensor_copy(outB[:], psumB[:])
        nc.scalar.dma_start(y_t.ap()[C * NCH : SPC, :], outB[:])

    nc.compile()
    return nc


def _host_params(v, g, b_fgl, W_fc, b_fc):
    """Fold the tiny params into zero-masked walm/warm [128, C*10], cc [1, C*10]."""
    v64 = v.astype(np.float64)
    w = g.astype(np.float64) * (v64[..., 0] / np.linalg.norm(v64, axis=-1))  # [4,4]
    A = np.einsum("qj,cqj->qc", w, W_fc.astype(np.float64).reshape(NCLS, 4, 4))
    cc = b_fgl.astype(np.float64).reshape(-1) @ W_fc.astype(np.float64).T
    cc = cc + b_fc.astype(np.float64)

    # quadrant ids: 0=TL, 1=BL, 2=BR, 3=TR
    def masks(pps, c):
        p = np.arange(128)
        top = (p % pps) < (pps // 2)
        al_col = np.where(top[:, None], A[0][None, :], A[1][None, :])
        ar_col = np.where(top[:, None], A[3][None, :], A[2][None, :])
        grp = p // pps
        wl = np.zeros((128, c * NCLS))
        wr = np.zeros((128, c * NCLS))
        for j in range(c):
            sel = grp == j
            wl[sel, j * NCLS : (j + 1) * NCLS] = al_col[sel]
            wr[sel, j * NCLS : (j + 1) * NCLS] = ar_col[sel]
        cb = np.tile(cc, c).reshape(1, c * NCLS)
        return (
            np.ascontiguousarray(wl, dtype=np.float32),
            np.ascontiguousarray(wr, dtype=np.float32),
            np.ascontiguousarray(cb, dtype=np.float32),
        )

    return masks(PPS, C), masks(128, 1)


def _run(inputs, trace=False):
    from concourse.bass_utils import run_bass_kernel_spmd

    if "nc" not in _PROGRAM_CACHE:
        _PROGRAM_CACHE["nc"] = _build_program()
    nc = _PROGRAM_CACHE["nc"]

    x = np.ascontiguousarray(np.asarray(inputs["x"], dtype=np.float32))
    (walm, warm, ccbt), (walm1, warm1, ccbt1) = _host_params(
        np.asarray(inputs["v"], np.float32),
        np.asarray(inputs["g"], np.float32),
        np.asarray(inputs["b_fgl"], np.float32),
        np.asarray(inputs["W_fc"], np.float32),
        np.asarray(inputs["b_fc"], np.float32),
    )

    cst = np.zeros((128, 150), np.float32)
    cst[:, 0:40] = walm
    cst[:, 40:80] = warm
    cst[:, 80:90] = walm1
    cst[:, 90:100] = warm1
    cst[0, 100:140] = ccbt[0]
    cst[0, 140:150] = ccbt1[0]
    x_sh = x.reshape(NCORES, SPC * S * S)
    nb = C * NCH * S * S  # floats in the bulk part
    in_maps = [
        {
            "x": x_sh[i, :nb].reshape(NCH, 128, FREE),
            "x1": x_sh[i, nb:].reshape(NT, 128, FREE1),
            "cst": cst,
        }
        for i in range(NCORES)
    ]
    res = run_bass_kernel_spmd(nc, in_maps, list(range(NCORES)), trace=trace)
    y = np.concatenate([res.results[i]["y"] for i in range(NCORES)], axis=0)
    return y, res.exec_time_ns


def kernel(**inputs) -> np.ndarray:
    y, _ = _run(inputs, trace=False)
    return y


# revision 12
# speedup vs baseline: 1.0049x; 1.0049x over previous
"""Trainium2 Bass kernel for nn_Classifier0 (quadrant-sum classifier).

reference:
    agg[n, q]  = quadrant sums of x[n] (512x512, quadrants of 256x256)
    w          = g * v[..., 0] / ||v||            [4, 4]
    y          = agg[:, :, None] * w + b_fgl      [N, 4, 4]
    out        = y.reshape(N, 16) @ W_fc.T + b_fc [N, 10]

Algebraic refactor (exact in real arithmetic):
    out[n, c] = sum_q agg[n, q] * A[q, c] + cc[c]
      A[q, c] = sum_j w[q, j] * W_fc[c, 4q + j]         (4 x 10, host, fp64)
      cc[c]   = b_fgl.ravel() @ W_fc[c] + b_fc[c]       (10, host, fp64)

Device work (data-parallel, 32 samples per core):
  - bulk: 7 chunks of C=4 samples, one contiguous 4 MB DMA each into a
    [128, 8192] tile (partition p holds 16 consecutive image rows of
    sample p // 32; p % 32 < 16 is the image's top half).
  - DVE tensor_reduce sums the left 256 columns of each row, ACT
    (in-place activation Copy with accum_out) sums the right 256
    -> bufL/bufR [128, 7].
  - the bulk quadrant contraction + fc (3 PSUM-accumulated matmuls with
    zero-masked weights; psum [7, 40] row-major equals y[0:28] with
    n = 4k + j) is placed BEFORE the tail loop so it fires as soon as
    chunk 6 is reduced (~80% through the stream) and its y write is
    hidden under the x stream.  y writes go out on the scalar (ACT)
    HWDGE ring so they never block tail x DMAs in the SP ring's FIFO.
  - tail: samples 28..31 as 4 single-sample 1 MB chunks keep the
    critical tail short (last reduce covers 1 MB, not 4 MB).

Per-core stream is SBUF-fabric bound: 16 SDMA engines x ~26.5 GB/s
= ~424 GB/s -> 33.6 MB in ~79 us, plus a ~8.6 us fixed NEFF preamble
(engine init, instruction loads, first-descriptor/HBM latency) and a
postamble of per-event EVENT_SEMAPHORE resets whose length scales with
the instruction/event count -- which is why chunk count is kept low.
"""

import numpy as np

N, S = 256, 512
H = S // 2
NCORES = 8
SPC = N // NCORES  # samples per core (32)
NCLS = 10

C = 4  # samples per DMA chunk (bulk)
NCH = 7  # bulk chunks per core (samples 0..27)
NT = 4  # single-sample tail chunks (samples 28..31)
PPS = 128 // C  # partitions per sample in a bulk chunk (32)
RPP = S // PPS  # image rows per partition (16)
FREE = S * RPP  # floats per partition per bulk chunk (8192)
FREE1 = S * 4  # floats per partition per C=1 chunk (2048)

_PROGRAM_CACHE = {}


def _build_program():
    from contextlib import ExitStack

    import concourse.bacc as bacc
    import concourse.mybir as mybir
    import concourse.tile as tile

    nc = bacc.Bacc("TRN2", target_bir_lowering=False, debug=False)
    dt = mybir.dt.float32
    dth = mybir.dt.float16

    x_t = nc.dram_tensor("x", [NCH, 128, FREE], dt, kind="ExternalInput")
    x1_t = nc.dram_tensor("x1", [NT, 128, FREE1], dt, kind="ExternalInput")
    # all folded params packed into one tensor: cols 0:40 walm, 40:80 warm,
    # 80:90 walm1, 90:100 warm1; row 0 cols 100:140 ccbt, 140:150 ccbt1
    cst_t = nc.dram_tensor("cst", [128, 150], dt, kind="ExternalInput")
    y_t = nc.dram_tensor("y", [SPC, NCLS], dt, kind="ExternalOutput")

    with tile.TileContext(nc) as tc, ExitStack() as ctx:
        xpool = ctx.enter_context(tc.tile_pool(name="xp", bufs=4))
        tpool = ctx.enter_context(tc.tile_pool(name="tp", bufs=4))
        cpool = ctx.enter_context(tc.tile_pool(name="cp", bufs=1))
        ppool = ctx.enter_context(tc.tile_pool(name="pp", bufs=1, space="PSUM"))

        x_ap = x_t.ap()
        x1_ap = x1_t.ap()
        # first 28 y rows viewed as [7 chunks, 40]
        y2 = y_t.ap()[0 : C * NCH, :].rearrange("(k j) c -> k (j c)", j=C)

        bufL = cpool.tile([128, NCH], dt)
        bufR = cpool.tile([128, NCH], dt)
        bufL1 = cpool.tile([128, NT], dt)
        bufR1 = cpool.tile([128, NT], dt)
        # one constant load on the scalar engine's HWDGE ring: the SP ring
        # starts streaming x immediately and GpSimd stays fully idle
        cst = cpool.tile([128, 150], dt)
        nc.scalar.dma_start(cst[:], cst_t.ap())
        walm, warm = cst[:, 0 : C * NCLS], cst[:, C * NCLS : 2 * C * NCLS]
        walm1, warm1 = cst[:, 80:90], cst[:, 90:100]
        ccbt, ccbt1 = cst[0:1, 100 : 100 + C * NCLS], cst[0:1, 140:150]
        ones1 = cpool.tile([1, NCH], dt)
        nc.vector.memset(ones1[:], 1.0)

        for k in range(NCH):
            xt = xpool.tile([128, FREE], dt)
            nc.sync.dma_start(xt[:], x_ap[k])
            xv = xt[:].rearrange("p (r c) -> p r c", c=S)
            nc.vector.tensor_reduce(
                bufL[:, k : k + 1],
                xv[:, :, 0:H],
                axis=mybir.AxisListType.XY,
                op=mybir.AluOpType.add,
            )
            nc.scalar.activation(
                xv[:, :, H:S],
                xv[:, :, H:S],
                mybir.ActivationFunctionType.Copy,
                accum_out=bufR[:, k : k + 1],
            )

        # bulk chunks: ready ~80% through the stream -- matmul + copy +
        # y write all hidden under the tail of the x stream
        psumA = ppool.tile([NCH, C * NCLS], dt)
        nc.tensor.matmul(psumA[:], lhsT=bufL[:], rhs=walm, start=True, stop=False)
        nc.tensor.matmul(psumA[:], lhsT=bufR[:], rhs=warm, start=False, stop=False)
        nc.tensor.matmul(psumA[:], lhsT=ones1[:], rhs=ccbt, start=False, stop=True)
        outA = cpool.tile([NCH, C * NCLS], dt)
        nc.vector.tensor_copy(outA[:], psumA[:])
        nc.scalar.dma_start(y2[:], outA[:])

        # single-sample tail chunks: half-size reduces on the critical tail
        for k in range(NT):
            xt1 = tpool.tile([128, FREE1], dt)
            nc.sync.dma_start(xt1[:], x1_ap[k])
            xv1 = xt1[:].rearrange("p (r c) -> p r c", c=S)
            nc.vector.tensor_reduce(
                bufL1[:, k : k + 1],
                xv1[:, :, 0:H],
                axis=mybir.AxisListType.XY,
                op=mybir.AluOpType.add,
            )
            nc.scalar.activation(
                xv1[:, :, H:S],
                xv1[:, :, H:S],
                mybir.ActivationFunctionType.Copy,
                accum_out=bufR1[:, k : k + 1],
            )

        # tail samples 28..31: the short critical tail
        ones2 = ones1[:, 0:NT]
        psumB = ppool.tile([NT, NCLS], dt)
        nc.tensor.matmul(psumB[:], lhsT=bufL1[:], rhs=walm1, start=True, stop=False)
        nc.tensor.matmul(psumB[:], lhsT=bufR1[:], rhs=warm1, start=False, stop=False)
        nc.tensor.matmul(psumB[:], lhsT=ones2, rhs=ccbt1, start=False, stop=True)
        outB = cpool.tile([NT, NCLS], dt)
        nc.vector.tensor_copy(outB[:], psumB[:])
        nc.scalar.dma_start(y_t.ap()[C * NCH : SPC, :], outB[:])

    nc.compile()
    return nc


def _host_params(v, g, b_fgl, W_fc, b_fc):
    """Fold the tiny params into zero-masked walm/warm [128, C*10], cc [1, C*10]."""
    v64 = v.astype(np.float64)
    w = g.astype(np.float64) * (v64[..., 0] / np.linalg.norm(v64, axis=-1))  # [4,4]
    A = np.einsum("qj,cqj->qc", w, W_fc.astype(np.float64).reshape(NCLS, 4, 4))
    cc = b_fgl.astype(np.float64).reshape(-1) @ W_fc.astype(np.float64).T
    cc = cc + b_fc.astype(np.float64)

    # quadrant ids: 0=TL, 1=BL, 2=BR, 3=TR
    def masks(pps, c):
        p = np.arange(128)
        top = (p % pps) < (pps // 2)
        al_col = np.where(top[:, None], A[0][None, :], A[1][None, :])
        ar_col = np.where(top[:, None], A[3][None, :], A[2][None, :])
        grp = p // pps
        wl = np.zeros((128, c * NCLS))
        wr = np.zeros((128, c * NCLS))
        for j in range(c):
            sel = grp == j
            wl[sel, j * NCLS : (j + 1) * NCLS] = al_col[sel]
            wr[sel, j * NCLS : (j + 1) * NCLS] = ar_col[sel]
        cb = np.tile(cc, c).reshape(1, c * NCLS)
        return (
            np.ascontiguousarray(wl, dtype=np.float32),
            np.ascontiguousarray(wr, dtype=np.float32),
            np.ascontiguousarray(cb, dtype=np.float32),
        )

    return masks(PPS, C), masks(128, 1)


def _run(inputs, trace=False):
    from concourse.bass_utils import run_bass_kernel_spmd

    if "nc" not in _PROGRAM_CACHE:
        _PROGRAM_CACHE["nc"] = _build_program()
    nc = _PROGRAM_CACHE["nc"]

    x = np.ascontiguousarray(np.asarray(inputs["x"], dtype=np.float32))
    (walm, warm, ccbt), (walm1, warm1, ccbt1) = _host_params(
        np.asarray(inputs["v"], np.float32),
        np.asarray(inputs["g"], np.float32),
        np.asarray(inputs["b_fgl"], np.float32),
        np.asarray(inputs["W_fc"], np.float32),
        np.asarray(inputs["b_fc"], np.float32),
    )

    cst = np.zeros((128, 150), np.float32)
    cst[:, 0:40] = walm
    cst[:, 40:80] = warm
    cst[:, 80:90] = walm1
    cst[:, 90:100] = warm1
    cst[0, 100:140] = ccbt[0]
    cst[0, 140:150] = ccbt1[0]
    x_sh = x.reshape(NCORES, SPC * S * S)
    nb = C * NCH * S * S  # floats in the bulk part
    in_maps = [
        {
            "x": x_sh[i, :nb].reshape(NCH, 128, FREE),
            "x1": x_sh[i, nb:].reshape(NT, 128, FREE1),
            "cst": cst,
        }
        for i in range(NCORES)
    ]
    res = run_bass_kernel_spmd(nc, in_maps, list(range(NCORES)), trace=trace)
    y = np.concatenate([res.results[i]["y"] for i in range(NCORES)], axis=0)
    return y, res.exec_time_ns


def kernel(**inputs) -> np.ndarray:
    y, _ = _run(inputs, trace=False)
    return y
